# revision 71
# baseline (speedup 1.0000x reference)
"""Trainium2 Bass kernel for AttentionWithRoPE (B=2, S=2048, HID=2048, H=16, D=128).

Sharding (8 cores): tensor-parallel over heads x data-parallel over batch.
Core c handles batch c//4 and heads 4*(c%4) .. 4*(c%4)+4.

Fully-fused software pipeline over 4 s-tile blocks; block j runs, back to
back on the PE: QK projection passes for s-tile j -> V projection for
s-tile j -> causal attention for q-tile j -> output-projection units of
block j-1. The interleave keeps the PE fed while ScalarE runs the softmax
exps and DVE runs RoPE / normalization of neighbouring stages.

  - QKV projection: fp8e4m3 DoubleRow matmuls (2 k-chunks/instruction,
    0.5 cyc/row) with 3-term error compensation; operands are split hi/lo
    on the HOST (x ~ x_hi + x_lo, both fp8; lo*lo term dropped, ~1e-3
    relative). Weights are pre-scaled by 64 so they sit mid-range in fp8;
    the descale rides the RoPE cos/sin tables (Q,K) and the PSUM->SBUF
    copy (V).
  - Attention per head in scores^T orientation ([k, q]); q/k/v/exp all
    fp16 in SBUF (same PE rate, 2x DVE, half the SBUF/DMA). exp on
    ScalarE from PSUM with 1/sqrt(D) folded in; fully-masked k-blocks
    skipped; diagonal blocks masked with an upper-tri fp16 tile. The
    softmax denominator: exp chunks are accumulated into an fp16 E_acc on
    DVE (2x all-fp16 adds), then ONE ones-MATRIX PE matmul reduces the
    128 partitions AND broadcasts the denominator to all partitions, so
    normalization is reciprocal (DVE) + one PSUM*SBUF multiply -- no
    separate broadcast matmul.
  - Output projection with the core's fp16 w_o column slice; the host
    sums the four fp16 partials per batch (the TP reduce).
"""
import numpy as np
import ml_dtypes
from contextlib import ExitStack

import concourse.bass as bass
import concourse.tile as tile
from concourse import bacc, mybir
from concourse.bass_utils import run_bass_kernel_spmd

B, S, HID = 2, 2048, 2048
H, D = 16, 128
NCORES = 8
NH = 4                 # heads per core
HC = HID // 128        # hid chunks
HCP = HC // 2          # hid chunk pairs (DoubleRow)
AST = 512              # s-tile width (both projection and q-tile)
NST = S // AST
QT = AST
NQT = NST
DSCALE = float(D) ** -0.5
ALPHA = 64.0           # host-side weight scale for fp8
ATS = 8.0              # attention-output scale for the fp8 at split
                       # (rides the ones matrix: rec = ATS/denominator)
F32 = mybir.dt.float32
F32R = mybir.dt.float32r
F16 = mybir.dt.float16
F8 = mybir.dt.float8e4
DR = mybir.MatmulPerfMode.DoubleRow

_CACHED = {}


def _build_nc():
    nc = bacc.Bacc("TRN2", target_bir_lowering=False, debug=False,
                   num_devices=NCORES)
    h_hi = nc.dram_tensor("h_hi", [HID, S], F8, kind="ExternalInput")
    h_lo = nc.dram_tensor("h_lo", [HID, S], F8, kind="ExternalInput")
    wqh = nc.dram_tensor("wqh", [HID, NH * D], F8, kind="ExternalInput")
    wql = nc.dram_tensor("wql", [HID, NH * D], F8, kind="ExternalInput")
    wkh = nc.dram_tensor("wkh", [HID, NH * D], F8, kind="ExternalInput")
    wkl = nc.dram_tensor("wkl", [HID, NH * D], F8, kind="ExternalInput")
    wvh = nc.dram_tensor("wvh", [HID, NH * D], F8, kind="ExternalInput")
    wvl = nc.dram_tensor("wvl", [HID, NH * D], F8, kind="ExternalInput")
    woh = nc.dram_tensor("woh", [NH * D, HID], F8, kind="ExternalInput")
    wol = nc.dram_tensor("wol", [NH * D, HID], F8, kind="ExternalInput")
    cosT = nc.dram_tensor("cosT", [D, S], F16, kind="ExternalInput")
    sinS = nc.dram_tensor("sinS", [D, S], F16, kind="ExternalInput")
    tri = nc.dram_tensor("tri", [128, 128], F16, kind="ExternalInput")
    ones = nc.dram_tensor("ones", [128, 128], F16, kind="ExternalInput")
    out = nc.dram_tensor("out", [S, HID], F16, kind="ExternalOutput")

    hh_r = h_hi.ap().rearrange("(hc p) s -> p hc s", p=128)
    hl_r = h_lo.ap().rearrange("(hc p) s -> p hc s", p=128)
    w_r = {name: t.ap().rearrange("(hc p) m -> p hc m", p=128)
           for name, t in (("wqh", wqh), ("wql", wql), ("wkh", wkh),
                           ("wkl", wkl), ("wvh", wvh), ("wvl", wvl))}
    wo_r = {"woh": woh.ap().rearrange("(g p) n -> p g n", p=128),
            "wol": wol.ap().rearrange("(g p) n -> p g n", p=128)}

    with tile.TileContext(nc) as tc, ExitStack() as ctx:
        # ---- constants / resident tensors ----
        constp = ctx.enter_context(tc.tile_pool(name="const", bufs=1))
        tri_sb = constp.tile([128, 128], F16, tag="tri", name="tri")
        ones_sb = constp.tile([128, 128], F16, tag="ones", name="ones")

        qkp = ctx.enter_context(tc.tile_pool(name="qk", bufs=1))
        qsb = qkp.tile([128, NH, S], F16, tag="qsb", name="qsb")
        ksb = qkp.tile([128, NH, S], F16, tag="ksb", name="ksb")
        vp = ctx.enter_context(tc.tile_pool(name="vp", bufs=1))
        v_sb = vp.tile([128, S // 128, NH * D], F16, tag="vsb", name="vsb")
        atp = ctx.enter_context(tc.tile_pool(name="at", bufs=1))
        at8h = atp.tile([128, NH, S], F8, tag="ath", name="ath")
        at8l = atp.tile([128, NH, S], F8, tag="atl", name="atl")
        wop = ctx.enter_context(tc.tile_pool(name="cwo", bufs=1))
        wo8h = wop.tile([128, NH, HID], F8, tag="woh", name="woh")
        wo8l = wop.tile([128, NH, HID], F8, tag="wol", name="wol")

        wp = ctx.enter_context(tc.tile_pool(name="w", bufs=1))
        wsb = {name: wp.tile([128, HC, NH * D], F8, tag=name, name=name)
               for name in ("wqh", "wql", "wkh", "wkl", "wvh", "wvl")}

        hpool = ctx.enter_context(tc.tile_pool(name="ah", bufs=2))
        cspool = ctx.enter_context(tc.tile_pool(name="acs", bufs=3))
        ropep = ctx.enter_context(tc.tile_pool(name="arope", bufs=1))
        expp = ctx.enter_context(tc.tile_pool(name="bexp", bufs=20))
        eaccp = ctx.enter_context(tc.tile_pool(name="beacc", bufs=3))
        smallp = ctx.enter_context(tc.tile_pool(name="bsmall", bufs=4))
        outp = ctx.enter_context(tc.tile_pool(name="cout", bufs=6))

        psA = ctx.enter_context(
            tc.tile_pool(name="psA", bufs=4, space="PSUM"))
        psS = ctx.enter_context(
            tc.tile_pool(name="psS", bufs=2, space="PSUM"))
        psW = ctx.enter_context(          # pv / csps / C units / V units
            tc.tile_pool(name="psW", bufs=2, space="PSUM"))

        def load_tile(st):
            sl = bass.ts(st, AST)
            hh = hpool.tile([128, HC, AST], F8, tag="hh", name="hh")
            hl = hpool.tile([128, HC, AST], F8, tag="hl", name="hl")
            cs_t = cspool.tile([128, AST], F16, tag="cs", name="cs")
            ss_t = cspool.tile([128, AST], F16, tag="ss", name="ss")
            if st == 0:
                # ordered so the first matmuls' inputs land first
                nc.sync.dma_start(wsb["wqh"][:, 0:2, :],
                                  w_r["wqh"][:, 0:2, :])
                nc.sync.dma_start(hh[:, 0:2, :], hh_r[:, 0:2, sl])
                nc.sync.dma_start(wsb["wqh"][:, 2:HC, :],
                                  w_r["wqh"][:, 2:HC, :])
                for c in range(1, 4):
                    nc.sync.dma_start(hh[:, 4 * c - 2:4 * c + 2, :],
                                      hh_r[:, 4 * c - 2:4 * c + 2, sl])
                nc.sync.dma_start(hh[:, HC - 2:HC, :],
                                  hh_r[:, HC - 2:HC, sl])
                nc.sync.dma_start(wsb["wql"][:], w_r["wql"])
                nc.sync.dma_start(tri_sb[:], tri.ap())
                nc.sync.dma_start(ones_sb[:], ones.ap())
                nc.sync.dma_start(cs_t[:], cosT.ap()[:, sl])
                nc.sync.dma_start(ss_t[:], sinS.ap()[:, sl])
                for c in range(4):
                    nc.sync.dma_start(hl[:, 4 * c:4 * c + 4, :],
                                      hl_r[:, 4 * c:4 * c + 4, sl])
                for name in ("wkh", "wkl", "wvh", "wvl"):
                    nc.sync.dma_start(wsb[name][:], w_r[name])
            else:
                for c in range(4):
                    nc.sync.dma_start(hh[:, 4 * c:4 * c + 4, :],
                                      hh_r[:, 4 * c:4 * c + 4, sl])
                nc.sync.dma_start(cs_t[:], cosT.ap()[:, sl])
                nc.sync.dma_start(ss_t[:], sinS.ap()[:, sl])
                for c in range(4):
                    nc.sync.dma_start(hl[:, 4 * c:4 * c + 4, :],
                                      hl_r[:, 4 * c:4 * c + 4, sl])
            return hh, hl, cs_t, ss_t

        cunits = [(sc, nt) for sc in range(S // 128)
                  for nt in range(HID // QT)]
        cpos_ref = [0]

        def emit_c_unit(sc, nt, flip, pool=None):
            # fp8 DoubleRow 3-term output projection: pairs of heads give
            # the 256-deep contraction; PSUM holds at*wo * ATS*ALPHA,
            # descaled in the copy out.
            ssl = bass.ts(sc, 128)
            nsl = bass.ts(nt, QT)
            ps = (pool or psW).tile([128, QT], F32, tag="w" if pool is None
                                    else "s", name="o")
            k = 0
            for a_t, w_t in ((at8h, wo8h), (at8l, wo8h), (at8h, wo8l)):
                for gp in range(NH // 2):
                    nc.tensor.matmul(
                        ps[:], a_t[:, 2 * gp:2 * gp + 2, ssl],
                        w_t[:, 2 * gp:2 * gp + 2, nsl],
                        start=(k == 0), stop=(k == 3 * (NH // 2) - 1),
                        perf_mode=DR,
                    )
                    k += 1
            ot = outp.tile([128, QT], F16, tag="ot", name="ot")
            if flip:
                nc.scalar.mul(ot[:], ps[:], 1.0 / (ATS * ALPHA))
            else:
                nc.vector.tensor_scalar(
                    ot[:], ps[:], 1.0 / (ATS * ALPHA), None,
                    mybir.AluOpType.mult)
            nc.sync.dma_start(out.ap()[ssl, nsl], ot[:])

        def emit_v_unit(sc, hh, hl):
            scl = slice((sc % 4) * 128, (sc % 4) * 128 + 128)
            ps = psW.tile([128, NH * D], F32, tag="w", name="vps")
            k = 0
            for wt, ht in ((wsb["wvh"], hh), (wsb["wvl"], hh),
                           (wsb["wvh"], hl)):
                for p in range(HCP):
                    nc.tensor.matmul(
                        ps[:], ht[:, 2 * p:2 * p + 2, scl],
                        wt[:, 2 * p:2 * p + 2, :],
                        start=(k == 0), stop=(k == 3 * HCP - 1),
                        perf_mode=DR,
                    )
                    k += 1
            if sc < 8:
                nc.scalar.mul(v_sb[:, sc, :], ps[:], 1.0 / ALPHA)
            else:
                nc.vector.tensor_scalar(
                    v_sb[:, sc, :], ps[:], 1.0 / ALPHA, None,
                    mybir.AluOpType.mult)

        nxt = load_tile(0)
        for st in range(NST):
            hh, hl, cs_t, ss_t = nxt
            sl = bass.ts(st, AST)
            cur_sl, cur_cs, cur_ss = sl, cs_t, ss_t

            # ---- QK projection passes for s-tile st ----
            def emit_rope(ps, dsb, h, sl=None, cs_t=None, ss_t=None):
                # RoPE: out = x*cos + shift(x)*sin_signed (descale by
                # 1/ALPHA folded into the host tables). ScalarE (idle in
                # projection sections) stages the PSUM result to SBUF fp16
                # straight + half-swapped, so every DVE op runs in the 2x
                # all-fp16 mode and the PSUM slot is released by ScalarE.
                sl = cur_sl if sl is None else sl
                cs_t = cur_cs if cs_t is None else cs_t
                ss_t = cur_ss if ss_t is None else ss_t
                swp = ropep.tile([128, AST], F16, tag="swp", name="swp")
                nc.scalar.copy(swp[0:64, :], ps[64:128, :])
                nc.scalar.copy(swp[64:128, :], ps[0:64, :])
                tsin = ropep.tile([128, AST], F16, tag="tsin",
                                  name="tsin")
                nc.vector.tensor_tensor(
                    tsin[:], swp[:], ss_t[:], mybir.AluOpType.mult)
                tcos = ropep.tile([128, AST], F16, tag="tcos",
                                  name="tcos")
                nc.vector.tensor_tensor(
                    tcos[:], ps[:], cs_t[:], mybir.AluOpType.mult)
                nc.vector.tensor_tensor(
                    dsb[:, h, sl], tcos[:], tsin[:],
                    mybir.AluOpType.add)

            def emit_term(ps, wt, ht, hsl, k0, last):
                for p in range(HCP):
                    nc.tensor.matmul(
                        ps[:], wt[:, 2 * p:2 * p + 2, hsl],
                        ht[:, 2 * p:2 * p + 2, :],
                        start=(k0 + p == 0), stop=(last and p == HCP - 1),
                        perf_mode=DR, skip_group_check=True,
                    )

            if st == 0:
                # term-major Q pass: all hi terms first (4 open PSUM
                # groups), so the PE is not blocked on h_lo's DMA arrival
                whi, wlo = wsb["wqh"], wsb["wql"]
                qps = []
                for h in range(NH):
                    hsl = slice(h * D, (h + 1) * D)
                    ps = psA.tile([128, AST], F32, tag="psqk", name="psqk")
                    qps.append(ps)
                    emit_term(ps, whi, hh, hsl, 0, False)
                    emit_term(ps, wlo, hh, hsl, HCP, False)
                for h in range(NH):
                    hsl = slice(h * D, (h + 1) * D)
                    emit_term(qps[h], whi, hl, hsl, 2 * HCP, True)
                    emit_rope(qps[h], qsb, h)
                passes = (("wk", ksb),)
            else:
                passes = (("wq", qsb), ("wk", ksb))

            for wn, dsb in passes:
                whi, wlo = wsb[wn + "h"], wsb[wn + "l"]
                for h in range(NH):
                    hsl = slice(h * D, (h + 1) * D)
                    ps = psA.tile([128, AST], F32, tag="psqk", name="psqk")
                    emit_term(ps, whi, hh, hsl, 0, False)
                    emit_term(ps, wlo, hh, hsl, HCP, False)
                    emit_term(ps, whi, hl, hsl, 2 * HCP, True)
                    emit_rope(ps, dsb, h)

            # prefetch next s-tile's hidden while attention runs
            if st + 1 < NST:
                nxt = load_tile(st + 1)
            if st == 1:
                for g in range(NH):
                    nc.sync.dma_start(wo8h[:, g, :], wo_r["woh"][:, g, :])
                for g in range(NH):
                    nc.sync.dma_start(wo8l[:, g, :], wo_r["wol"][:, g, :])

            # ---- V projection for s-tile st ----
            for sc in range(4 * st, 4 * st + 4):
                emit_v_unit(sc, hh, hl)

            # ---- attention block qt = st ----
            def attn_chunks(qt, h, kc0, kc1, eacc, eacc_init, pvps,
                            pv_start, nunits):
                # scores^T/exp/denominator-accumulate for chunks
                # [kc0, kc1), then the PV accumulation over them
                cslots = {kc0 + ((k + 1) * (kc1 - kc0)) // (nunits + 1)
                          for k in range(nunits)}
                ebs = {}
                for kc in range(kc0, kc1):
                    j = kc - 4 * qt
                    lo = max(0, 128 * j)
                    sps = psS.tile([128, QT], F32, tag="s", name="s")
                    eb = expp.tile([128, QT], F16, tag="e", name="e")
                    ebs[kc] = eb
                    nc.tensor.matmul(
                        sps[:, lo:QT],
                        ksb[:, h, kc * 128:(kc + 1) * 128],
                        qsb[:, h, qt * QT + lo:(qt + 1) * QT],
                        start=True, stop=True,
                    )
                    nc.scalar.activation(
                        eb[:, lo:QT], sps[:, lo:QT],
                        mybir.ActivationFunctionType.Exp, scale=DSCALE)
                    if j >= 0:
                        nc.vector.tensor_tensor(
                            eb[:, lo:lo + 128], eb[:, lo:lo + 128],
                            tri_sb[:], mybir.AluOpType.mult)
                    if kc == kc0 and eacc_init:
                        nc.vector.tensor_copy(eacc[:, lo:QT], eb[:, lo:QT])
                    else:
                        nc.vector.tensor_tensor(
                            eacc[:, lo:QT], eb[:, lo:QT], eacc[:, lo:QT],
                            mybir.AluOpType.add)
                    if kc in cslots:
                        csc, cnt = cunits[cpos_ref[0]]
                        emit_c_unit(csc, cnt, cpos_ref[0] < 28)
                        cpos_ref[0] += 1
                for kc in range(kc0, kc1):
                    lo = max(0, 128 * (kc - 4 * qt))
                    nc.tensor.matmul(
                        pvps[:, lo:QT],
                        v_sb[:, kc, h * D:(h + 1) * D],
                        ebs[kc][:, lo:QT],
                        start=(kc == kc0 and pv_start),
                        stop=(kc == kc1 - 1),
                        skip_group_check=True,
                    )

            def attn_finish(qt, h, eacc, pvps, nunits):
                qsl = bass.ts(qt, QT)
                csps = psW.tile([128, QT], F32, tag="w", name="cs")
                # denominator: ones-matrix lhsT (valued 1/ATS) reduces the
                # partitions AND broadcasts den/ATS to all 128 partitions
                nc.tensor.matmul(csps[:], ones_sb[:], eacc[:],
                                 start=True, stop=True)
                rec = smallp.tile([128, QT], F16, tag="rec", name="rec")
                with nc.allow_low_precision(
                        reason="softmax denom reciprocal to fp16"):
                    nc.vector.reciprocal(rec[:], csps[:])
                # normalized (and ATS-scaled) attention output, split hi/lo
                # to fp8 for the DoubleRow output projection
                atf = smallp.tile([128, QT], F16, tag="atf", name="atf")
                nc.vector.tensor_tensor(
                    atf[:], pvps[:], rec[:], mybir.AluOpType.mult)
                nc.vector.tensor_copy(at8h[:, h, qsl], atf[:])
                nc.vector.tensor_tensor(
                    at8l[:, h, qsl], atf[:], at8h[:, h, qsl],
                    mybir.AluOpType.subtract)
                for u in range(nunits):
                    csc, cnt = cunits[cpos_ref[0]]
                    emit_c_unit(csc, cnt, cpos_ref[0] < 28)
                    cpos_ref[0] += 1

            qt = st
            for h in range(NH):
                nallow = 4 * qt + 4
                pvps = psW.tile([128, QT], F32, tag="w", name="pv")
                eacc = eaccp.tile([128, QT], F16, tag="ea", name="ea")
                attn_chunks(qt, h, 0, nallow, eacc, True, pvps, True,
                            3 if qt > 0 else 0)
                attn_finish(qt, h, eacc, pvps, 1 if qt > 0 else 0)

        # remaining output-projection units (last block); alternate the
        # (now idle) psS pool with psW for double pipeline depth
        while cpos_ref[0] < len(cunits):
            sc, nt = cunits[cpos_ref[0]]
            emit_c_unit(sc, nt, cpos_ref[0] % 2 == 0,
                        pool=psS if cpos_ref[0] % 2 == 0 else None)
            cpos_ref[0] += 1

    nc.compile()
    return nc


def _fp8_split(x):
    hi = x.astype(ml_dtypes.float8_e4m3)
    lo = (x - hi.astype(np.float32)).astype(ml_dtypes.float8_e4m3)
    return (np.ascontiguousarray(hi).view(np.uint8),
            np.ascontiguousarray(lo).view(np.uint8))


def _prep_in_maps(hidden_states, cos, sin, w_qkv, w_o):
    hs = np.ascontiguousarray(np.asarray(hidden_states, dtype=np.float32))
    cos = np.asarray(cos, dtype=np.float32)
    sin = np.asarray(sin, dtype=np.float32)
    w_qkv = np.asarray(w_qkv, dtype=np.float32)
    w_o = np.asarray(w_o, dtype=np.float32)

    wT = np.ascontiguousarray(w_qkv.T) * ALPHA   # (HID, 3*H*D), pre-scaled
    woTf = np.ascontiguousarray(w_o.T)           # (H*D, HID)
    cosT = (np.ascontiguousarray(cos.T) / ALPHA).astype(np.float16)
    sinT = np.ascontiguousarray(sin.T)
    sinS = sinT.copy()
    sinS[:64] = -sinT[:64]
    sinS = (sinS / ALPHA).astype(np.float16)
    tri = np.triu(np.ones((128, 128), np.float16))
    ones = np.full((128, 128), 1.0 / ATS, np.float16)

    h_split = [_fp8_split(np.ascontiguousarray(hs[b].T)) for b in range(B)]
    w_split = []                                 # per head-group hi/lo
    for hg in range(4):
        lo_, hi_ = hg * NH * D, (hg + 1) * NH * D
        w_split.append({
            "wq": _fp8_split(np.ascontiguousarray(wT[:, lo_:hi_])),
            "wk": _fp8_split(np.ascontiguousarray(
                wT[:, H * D + lo_:H * D + hi_])),
            "wv": _fp8_split(np.ascontiguousarray(
                wT[:, 2 * H * D + lo_:2 * H * D + hi_])),
            "wo": _fp8_split(np.ascontiguousarray(woTf[lo_:hi_, :]) * ALPHA),
        })

    in_maps = []
    for c in range(NCORES):
        b, hg = c // 4, c % 4
        ws = w_split[hg]
        in_maps.append({
            "h_hi": h_split[b][0],
            "h_lo": h_split[b][1],
            "wqh": ws["wq"][0], "wql": ws["wq"][1],
            "wkh": ws["wk"][0], "wkl": ws["wk"][1],
            "wvh": ws["wv"][0], "wvl": ws["wv"][1],
            "woh": ws["wo"][0], "wol": ws["wo"][1],
            "cosT": cosT,
            "sinS": sinS,
            "tri": tri,
            "ones": ones,
        })
    return in_maps


def kernel(hidden_states, cos, sin, w_qkv, w_o, _trace=False):
    if "nc" not in _CACHED:
        _CACHED["nc"] = _build_nc()
    nc = _CACHED["nc"]
    in_maps = _prep_in_maps(hidden_states, cos, sin, w_qkv, w_o)
    res = run_bass_kernel_spmd(nc, in_maps, core_ids=list(range(NCORES)),
                               trace=_trace)
    _CACHED["last_result"] = res
    out = np.zeros((B, S, HID), np.float32)
    for c in range(NCORES):
        out[c // 4] += res.results[c]["out"]
    return out
